# revision 8
# baseline (speedup 1.0000x reference)
"""Trainium2 Bass kernel for a batched binary-tree (child-sum-ish) LSTM cell.

Computes, for N=8192 nodes (D = HD = 1024):
    z   = sigmoid([x_l x_r] @ W_z.T + b_z)
    x_t = z * x_l + (1-z) * x_r
    [x_i x_f x_o x_g] = x_t @ W_xin.T
    i   = sigmoid([h_l h_r c_l c_r] @ W_i.T  + b_i  + x_i)
    f_l = sigmoid([h_l h_r c_l c_r] @ W_fl.T + b_fl + x_f)
    f_r = sigmoid([h_l h_r c_l c_r] @ W_fr.T + b_fr + x_f)
    g   = tanh   ([h_l h_r]         @ W_g.T  + b_g  + x_g)
    c_t = f_l*c_l + f_r*c_r + i*g
    o   = sigmoid([h_l h_r c_t]     @ W_o.T  + b_o  + x_o)
    h_t = o * tanh(c_t)
returns (x_t, h_t, c_t).

Strategy: data-parallel over 8 NeuronCores (NL=1024 rows each), transposed
space (features on partitions, rows on the free dim). All activations stay
SBUF-resident, processed as two 512-row chunks (PSUM bank = 512 fp32), so
every weight tile streams from HBM exactly once. Matmuls in fp16 (8x less
rounding than bf16, same PE speed), fp32 PSUM/elementwise.

Ramp engineering: phase A (z gate) runs k-outer over all 8 m-PSUMs with a
kt-major weight layout and per-k-block input loads on the HWDGE rings, so
the first matmul fires as soon as ~0.3MB lands and the DMA stream never
gets ahead-of-use; a short burst of scratch matmuls during the initial DMA
latency window warms the PE HAM clock gate. Cross-phase inputs (xtb, ctb)
sit late in each gate's K-accumulation so phase tails hide under
dependency-free matmuls; the last output tile is processed in 256-column
halves to overlap its activation/store chain.
"""

import sys

if "/opt/trn_rl_repo" not in sys.path:
    sys.path.insert(0, "/opt/trn_rl_repo")

import numpy as np

N_CORES = 8
N = 8192
D = 1024
P = 128
NL = N // N_CORES          # rows per core
KB = D // P                # 8 k-blocks per 1024-feature tensor
MT = D // P                # 8 output m-tiles per gate
R = 512                    # rows per matmul chunk (PSUM bank = 512 fp32)
NCH = NL // R              # 2 chunks
N_WARM = 8                 # scratch matmuls to ramp the HAM clock gate

# (name, K-tiles, bias index, act fn, [(rhs part, weight k-tile offset)]).
# Part order: h's first (streaming in), xtb mid (hides phase A's tail), c's
# late (their DMA can trail the h's).
_GATES_B = [
    ("i",  40, 1, "sig",  [("hbl", 0), ("hbr", 8), ("xtb", 32),
                           ("cbl", 16), ("cbr", 24)], None),
    ("xf", 8,  None, None, [("xtb", 0)], None),
    ("fl", 32, 2, "sig",  [("hbl", 0), ("hbr", 8), ("cbl", 16), ("cbr", 24)], "xf"),
    ("fr", 32, 3, "sig",  [("hbl", 0), ("hbr", 8), ("cbl", 16), ("cbr", 24)], "xf"),
    ("g",  24, 4, "tanh", [("hbl", 0), ("hbr", 8), ("xtb", 16)], None),
]

_compiled = {}


def _build():
    """Build + compile the per-core Bass program."""
    import concourse.mybir as mybir
    import concourse.tile as tile
    from concourse import bacc

    F32 = mybir.dt.float32
    F16 = mybir.dt.float16
    SIG = mybir.ActivationFunctionType.Sigmoid
    TANH = mybir.ActivationFunctionType.Tanh

    nc = bacc.Bacc("TRN2", target_bir_lowering=False, debug=False)

    def din(name, shape, dt):
        return nc.dram_tensor(name, shape, dt, kind="ExternalInput").ap()

    def dout(name, shape, dt):
        return nc.dram_tensor(name, shape, dt, kind="ExternalOutput").ap()

    # Transposed activations [D, NL], fp16.
    xb_l = din("xb_l", [D, NL], F16)
    xb_r = din("xb_r", [D, NL], F16)
    hb_l = din("hb_l", [D, NL], F16)
    hb_r = din("hb_r", [D, NL], F16)
    cb_l = din("cb_l", [D, NL], F16)
    cb_r = din("cb_r", [D, NL], F16)
    # z weight kt-major [Kt, P, MT, P] for the k-outer phase A; the rest
    # m-major [MT, P, Kt, P] (partition-major: per-partition DMA runs are
    # Kt*256B / MT*256B contiguous).
    wzk = din("wzk", [16, P, MT, P], F16)
    wi = din("wi", [MT, P, 40, P], F16)
    wfl = din("wfl", [MT, P, 32, P], F16)
    wfr = din("wfr", [MT, P, 32, P], F16)
    wg = din("wg", [MT, P, 24, P], F16)
    wo = din("wo", [MT, P, 32, P], F16)
    wxf = din("wxf", [MT, P, 8, P], F16)
    wmap = {"i": wi, "xf": wxf, "fl": wfl, "fr": wfr, "g": wg}
    bias = din("bias", [P, 6, MT], F32)

    xT_o = dout("xT_o", [D, NL], F32)
    hT_o = dout("hT_o", [D, NL], F32)
    cT_o = dout("cT_o", [D, NL], F32)

    def r3(ap):
        return ap.rearrange("(k p) n -> p k n", p=P)

    with tile.TileContext(nc) as tc:
        with (
            tc.tile_pool(name="acts", bufs=1) as apool,
            tc.tile_pool(name="w", bufs=2) as wpool,
            tc.tile_pool(name="wz", bufs=6) as wzpool,
            tc.tile_pool(name="gates", bufs=10) as gpool,
            tc.tile_pool(name="work", bufs=8) as wkpool,
            tc.tile_pool(name="ps", bufs=8, space="PSUM") as pspool,
            tc.tile_pool(name="cst", bufs=1) as cpool,
        ):
            bias_t = cpool.tile([P, 6, MT], F32, name="bias_t")
            nc.sync.dma_start(bias_t[:], bias[:])

            # Scratch warm-up matmuls: no DMA dependency, so they run during
            # the initial descriptor/transfer latency and lift the PE HAM
            # clock gate to 2.4GHz before real matmuls arrive.
            wu_w = cpool.tile([P, P], F16, name="wu_w")
            nc.vector.memset(wu_w[:], 0.0)
            wu_m = cpool.tile([P, R], F16, name="wu_m")
            nc.vector.memset(wu_m[:], 0.0)
            wu_ps = pspool.tile([P, R], F32, tag="ps", name="wu_ps")
            for _ in range(N_WARM):
                nc.tensor.matmul(wu_ps[:], wu_w[:], wu_m[:],
                                 start=True, stop=True)



            # x inputs ride the HWDGE scalar ring (stores are idle early;
            # the SWDGE ring takes ~4us to spin up), one DMA per k-block in
            # consumption order.
            xbl_t = apool.tile([P, KB, NL], F16, tag="xbl", name="xbl")
            xbr_t = apool.tile([P, KB, NL], F16, tag="xbr", name="xbr")
            for j in range(KB):
                nc.scalar.dma_start(xbl_t[:, j, :], r3(xb_l)[:, j, :])
            for j in range(KB):
                nc.scalar.dma_start(xbr_t[:, j, :], r3(xb_r)[:, j, :])

            def lda(name, dram):
                t = apool.tile([P, KB, NL], F16, tag=name, name=name)
                nc.gpsimd.dma_start(t[:], r3(dram)[:])
                return t

            hbl_t = lda("hbl", hb_l)
            hbr_t = lda("hbr", hb_r)
            cbl_t = lda("cbl", cb_l)
            cbr_t = lda("cbr", cb_r)

            xtb_t = apool.tile([P, KB, NL], F16, tag="xtb", name="xtb")
            ctb_t = apool.tile([P, KB, NL], F16, tag="ctb", name="ctb")
            parts = {"hbl": hbl_t, "hbr": hbr_t, "cbl": cbl_t,
                     "cbr": cbr_t, "xtb": xtb_t, "ctb": ctb_t}

            def chunks():
                return [slice(c * R, (c + 1) * R) for c in range(NCH)]

            # ---- Phase A: z gate + x_t ----
            # k-outer over m-groups of 2 (4 PSUM banks per group, so two
            # groups pipeline in the 8-bank ring) with a small ring of
            # per-kt weight slices: the first matmul fires after ~0.3MB of
            # DMA, and x streams in k-block order with no ahead-of-use.
            MG = 2
            for g in range(MT // MG):
                ms = [g * MG + mi for mi in range(MG)]
                pss = [[pspool.tile([P, R], F32, tag="ps", name=f"ps_z{m}")
                        for _ in range(NCH)] for m in ms]
                for kt in range(16):
                    wslc = wzpool.tile([P, MG, P], F16, tag="wz",
                                       name="wz_slc")
                    nc.sync.dma_start(
                        wslc[:], wzk[kt][:, g * MG:(g + 1) * MG, :])
                    xsrc = (xbl_t if kt < KB else xbr_t)[:, kt % KB, :]
                    for mi in range(MG):
                        for ci, cs in enumerate(chunks()):
                            nc.tensor.matmul(pss[mi][ci][:], wslc[:, mi, :],
                                             xsrc[:, cs],
                                             start=(kt == 0), stop=(kt == 15))
                for mi in range(MG):
                    m = ms[mi]
                    for ci, cs in enumerate(chunks()):
                        z_t = wkpool.tile([P, R], F32, tag="wk", name="z_t")
                        nc.scalar.activation(z_t[:], pss[mi][ci][:], SIG,
                                             bias=bias_t[:, 0, m, None])
                        d_t = wkpool.tile([P, R], F32, tag="wk", name="d_t")
                        nc.vector.tensor_sub(d_t[:], xbl_t[:, m, cs],
                                             xbr_t[:, m, cs])
                        xrf_m = wkpool.tile([P, R], F32, tag="wk",
                                            name="xrf_m")
                        nc.vector.tensor_copy(xrf_m[:], xbr_t[:, m, cs])
                        nc.vector.tensor_mul(d_t[:], d_t[:], z_t[:])
                        xt_m = wkpool.tile([P, R], F32, tag="wk", name="xt_m")
                        nc.vector.tensor_add(xt_m[:], d_t[:], xrf_m[:])
                        nc.scalar.dma_start(r3(xT_o)[:, m, cs], xt_m[:])
                        nc.vector.tensor_copy(xtb_t[:, m, cs], xt_m[:])

            # ---- Phase B: i, f_l, f_r, g gates + c_t ----
            for m in range(MT):
                gt = {}   # gate -> per-chunk fp32 tiles
                xfp = []
                for (gname, Kt, b_idx, fn, rparts, xkey) in _GATES_B:
                    w_t = wpool.tile([P, Kt, P], F16, tag="w",
                                     name=f"w_{gname}")
                    nc.sync.dma_start(w_t[:], wmap[gname][m])
                    per_chunk = []
                    for cs in chunks():
                        ps = pspool.tile([P, R], F32, tag="ps",
                                         name=f"ps_{gname}")
                        n_done = 0
                        for (pname, koff) in rparts:
                            pt = parts[pname]
                            for j in range(KB):
                                nc.tensor.matmul(
                                    ps[:], w_t[:, koff + j, :], pt[:, j, cs],
                                    start=(n_done == 0),
                                    stop=(n_done == Kt - 1))
                                n_done += 1
                        if gname == "xf":
                            xf_c = gpool.tile([P, R], F32, tag="gate",
                                              name="xfp")
                            nc.scalar.copy(xf_c[:], ps[:])
                            xfp.append(xf_c)
                            continue
                        if xkey == "xf":
                            nc.vector.tensor_add(ps[:], ps[:],
                                                 xfp[len(per_chunk)][:])
                        g_t = gpool.tile([P, R], F32, tag="gate",
                                         name=f"g_{gname}")
                        nc.scalar.activation(
                            g_t[:], ps[:], SIG if fn == "sig" else TANH,
                            bias=bias_t[:, b_idx, m, None])
                        per_chunk.append(g_t)
                    if gname != "xf":
                        gt[gname] = per_chunk
                for ci, cs in enumerate(chunks()):
                    cfl_m = wkpool.tile([P, R], F32, tag="wk", name="cfl_m")
                    nc.vector.tensor_copy(cfl_m[:], cbl_t[:, m, cs])
                    cfr_m = wkpool.tile([P, R], F32, tag="wk", name="cfr_m")
                    nc.vector.tensor_copy(cfr_m[:], cbr_t[:, m, cs])
                    ct_m = wkpool.tile([P, R], F32, tag="wk", name="ct_m")
                    nc.vector.tensor_mul(ct_m[:], gt["fl"][ci][:], cfl_m[:])
                    t2 = wkpool.tile([P, R], F32, tag="wk", name="t2")
                    nc.vector.tensor_mul(t2[:], gt["fr"][ci][:], cfr_m[:])
                    nc.vector.tensor_add(ct_m[:], ct_m[:], t2[:])
                    nc.vector.tensor_mul(t2[:], gt["i"][ci][:], gt["g"][ci][:])
                    nc.vector.tensor_add(ct_m[:], ct_m[:], t2[:])
                    nc.scalar.dma_start(r3(cT_o)[:, m, cs], ct_m[:])
                    nc.vector.tensor_copy(ctb_t[:, m, cs], ct_m[:])

            # ---- Phase C: o gate + h_t ---- (ctb last in K so phase B's
            # tail hides under the hbl/hbr/xtb matmuls)
            o_parts = [("hbl", 0), ("hbr", 8), ("xtb", 24), ("ctb", 16)]
            tct7 = cpool.tile([P, NCH, R], F32, name="tct7")
            for m in range(MT):
                w_t = wpool.tile([P, 32, P], F16, tag="w", name="wo_t")
                nc.sync.dma_start(w_t[:], wo[m])
                last = (m == MT - 1)
                for ci, cs in enumerate(chunks()):
                    # The very last chunk is processed in 256-col halves so
                    # its activation/store chain overlaps its own matmuls.
                    halves = ([slice(0, 256), slice(256, 512)]
                              if (last and ci == NCH - 1) else [slice(0, R)])
                    for hs in halves:
                        rh = hs.stop - hs.start
                        rs = slice(cs.start + hs.start, cs.start + hs.stop)
                        ps = pspool.tile([P, rh], F32, tag="ps", name="ps_o")
                        kt = 0
                        for pname, koff in o_parts:
                            pt = parts[pname]
                            for j in range(KB):
                                nc.tensor.matmul(ps[:], w_t[:, koff + j, :],
                                                 pt[:, j, rs],
                                                 start=(kt == 0),
                                                 stop=(kt == 31))
                                kt += 1
                        o_t = wkpool.tile([P, rh], F32, tag="wk", name="o_t")
                        nc.scalar.activation(o_t[:], ps[:], SIG,
                                             bias=bias_t[:, 5, m, None])
                        if last:
                            tct_v = tct7[:, ci, hs]
                        else:
                            tct_m = wkpool.tile([P, rh], F32, tag="wk",
                                                name="tct_m")
                            nc.scalar.activation(tct_m[:], ctb_t[:, m, rs],
                                                 TANH)
                            tct_v = tct_m[:]
                        ht_m = wkpool.tile([P, rh], F32, tag="wk", name="ht_m")
                        nc.vector.tensor_mul(ht_m[:], o_t[:], tct_v)
                        nc.scalar.dma_start(r3(hT_o)[:, m, rs], ht_m[:])
                if m == 0:
                    # Precompute the last m-tile's tanh(c) while its ctb is
                    # long since ready, off the critical tail.
                    for ci, cs in enumerate(chunks()):
                        nc.scalar.activation(tct7[:, ci, :],
                                             ctb_t[:, MT - 1, cs], TANH)

    nc.compile()
    return nc


def _get_compiled():
    if "k" not in _compiled:
        _compiled["k"] = _build()
    return _compiled["k"]


def _prep_weight(w_km):
    """[K, D] (K-major stack of W.T blocks) -> [MT, P, Kt, P] fp16."""
    K = w_km.shape[0]
    kt = K // P
    w = w_km.reshape(kt, P, MT, P)          # [kt, p, m, f]
    w = np.ascontiguousarray(w.transpose(2, 1, 0, 3))  # [m, p, kt, f]
    return w.astype(np.float16)


def _host_prep(inp):
    """Transpose/stack/cast everything the device program wants."""
    f32 = np.float32
    t = {k: np.asarray(inp[k], dtype=f32).T.astype(np.float16)
         for k in ("x_l", "x_r", "h_l", "h_r", "c_l", "c_r")}

    W_i = np.asarray(inp["W_i"], f32)
    W_fl = np.asarray(inp["W_fl"], f32)
    W_fr = np.asarray(inp["W_fr"], f32)
    W_xin = np.asarray(inp["W_xin"], f32)
    W_o = np.asarray(inp["W_o"], f32)
    W_z = np.asarray(inp["W_z"], f32)
    W_g = np.asarray(inp["W_g"], f32)

    x_i = W_xin[0 * D:1 * D].T    # [D, D] blocks of W_xin.T
    x_f = W_xin[1 * D:2 * D].T
    x_o = W_xin[2 * D:3 * D].T
    x_g = W_xin[3 * D:4 * D].T

    # kt-major z weight for the k-outer phase A: [Kt, P, MT, P].
    wz_km = np.ascontiguousarray(W_z.T)               # [2048, 1024]
    wzk = np.ascontiguousarray(wz_km.reshape(16, P, MT, P)).astype(np.float16)

    # Device-side part offsets:
    #   wi: [h_l h_r c_l c_r | x_i]  (hbl 0, hbr 8, cbl 16, cbr 24, xtb 32)
    #   wg: [h_l h_r | x_g]          (hbl 0, hbr 8, xtb 16)
    #   wo: [h_l h_r c_t | x_o]      (hbl 0, hbr 8, ctb 16, xtb 24)
    weights = {
        "wzk": wzk,
        "wi": _prep_weight(np.concatenate([W_i.T, x_i], axis=0)),
        "wfl": _prep_weight(np.ascontiguousarray(W_fl.T)),
        "wfr": _prep_weight(np.ascontiguousarray(W_fr.T)),
        "wg": _prep_weight(np.concatenate([W_g.T, x_g], axis=0)),
        "wo": _prep_weight(np.concatenate([W_o.T, x_o], axis=0)),
        "wxf": _prep_weight(x_f),
    }

    b = np.stack([np.asarray(inp[k], f32) for k in
                  ("b_z", "b_i", "b_fl", "b_fr", "b_g", "b_o")])  # [6, D]
    bias = np.ascontiguousarray(b.reshape(6, MT, P).transpose(2, 0, 1))

    in_maps = []
    for c in range(N_CORES):
        cs = slice(c * NL, (c + 1) * NL)
        m = {
            "xb_l": np.ascontiguousarray(t["x_l"][:, cs]),
            "xb_r": np.ascontiguousarray(t["x_r"][:, cs]),
            "hb_l": np.ascontiguousarray(t["h_l"][:, cs]),
            "hb_r": np.ascontiguousarray(t["h_r"][:, cs]),
            "cb_l": np.ascontiguousarray(t["c_l"][:, cs]),
            "cb_r": np.ascontiguousarray(t["c_r"][:, cs]),
            "bias": bias,
        }
        m.update(weights)
        in_maps.append(m)
    return in_maps


def run(inputs, trace=False, trace_kwargs=None):
    """Run on 8 cores; returns (results, BassKernelResults)."""
    from concourse.bass_utils import run_bass_kernel_spmd

    if trace:
        try:
            from hookfix import install_ntff_hook
            install_ntff_hook()
        except Exception:
            pass
    nc = _get_compiled()
    in_maps = _host_prep(inputs)
    res = run_bass_kernel_spmd(nc, in_maps, core_ids=list(range(N_CORES)),
                               trace=trace, **(trace_kwargs or {}))
    xT = np.concatenate([res.results[c]["xT_o"] for c in range(N_CORES)], axis=1)
    hT = np.concatenate([res.results[c]["hT_o"] for c in range(N_CORES)], axis=1)
    cT = np.concatenate([res.results[c]["cT_o"] for c in range(N_CORES)], axis=1)
    x_t = np.ascontiguousarray(xT.T)
    h_t = np.ascontiguousarray(hT.T)
    c_t = np.ascontiguousarray(cT.T)
    return (x_t, h_t, c_t), res


def kernel(**inputs):
    out, _ = run(inputs)
    return out


# revision 9
# speedup vs baseline: 1.0254x; 1.0254x over previous
"""Trainium2 Bass kernel for a batched binary-tree (child-sum-ish) LSTM cell.

Computes, for N=8192 nodes (D = HD = 1024):
    z   = sigmoid([x_l x_r] @ W_z.T + b_z)
    x_t = z * x_l + (1-z) * x_r
    [x_i x_f x_o x_g] = x_t @ W_xin.T
    i   = sigmoid([h_l h_r c_l c_r] @ W_i.T  + b_i  + x_i)
    f_l = sigmoid([h_l h_r c_l c_r] @ W_fl.T + b_fl + x_f)
    f_r = sigmoid([h_l h_r c_l c_r] @ W_fr.T + b_fr + x_f)
    g   = tanh   ([h_l h_r]         @ W_g.T  + b_g  + x_g)
    c_t = f_l*c_l + f_r*c_r + i*g
    o   = sigmoid([h_l h_r c_t]     @ W_o.T  + b_o  + x_o)
    h_t = o * tanh(c_t)
returns (x_t, h_t, c_t).

Strategy: data-parallel over 8 NeuronCores (NL=1024 rows each), transposed
space (features on partitions, rows on the free dim). All activations stay
SBUF-resident, processed as two 512-row chunks (PSUM bank = 512 fp32), so
every weight tile streams from HBM once (z weight twice, once per chunk
pass). Matmuls in fp16 (8x less rounding than bf16, same PE speed), fp32
PSUM/elementwise.

Ramp engineering: all 8 cores start pulling HBM at t=0, so the chip-wide
DMA fabric is the scarce resource for the first ~30us. Phase A (z) runs
k-outer over all 8 m-PSUMs per chunk pass, consuming x and the kt-major z
weight slices in stream order at ~300GB/s; everything not needed in that
window (h, c loads) is gated behind phase-A progress via tiny gpsimd
copies, so it cannot crowd the critical stream. A short scratch-matmul
burst bridges the initial DMA latency and lifts the PE HAM clock gate.
Cross-phase inputs (xtb, ctb) sit late in each gate's K-accumulation so
phase tails hide under dependency-free matmuls; the last output tile is
processed in 256-column halves to overlap its activation/store chain.
"""

import sys

if "/opt/trn_rl_repo" not in sys.path:
    sys.path.insert(0, "/opt/trn_rl_repo")

import numpy as np

N_CORES = 8
N = 8192
D = 1024
P = 128
NL = N // N_CORES          # rows per core
KB = D // P                # 8 k-blocks per 1024-feature tensor
MT = D // P                # 8 output m-tiles per gate
R = 512                    # rows per matmul chunk (PSUM bank = 512 fp32)
NCH = NL // R              # 2 chunks
N_WARM = 16                # scratch matmuls to ramp the HAM clock gate

# (name, K-tiles, bias index, act fn, [(rhs part, weight k-tile offset)]).
# Part order: h's first (streaming in), xtb mid (hides phase A's tail), c's
# late (their DMA trails the h's).
_GATES_B = [
    ("i",  40, 1, "sig",  [("hbl", 0), ("hbr", 8), ("xtb", 32),
                           ("cbl", 16), ("cbr", 24)], None),
    ("xf", 8,  None, None, [("xtb", 0)], None),
    ("fl", 32, 2, "sig",  [("hbl", 0), ("hbr", 8), ("cbl", 16), ("cbr", 24)], "xf"),
    ("fr", 32, 3, "sig",  [("hbl", 0), ("hbr", 8), ("cbl", 16), ("cbr", 24)], "xf"),
    ("g",  24, 4, "tanh", [("hbl", 0), ("hbr", 8), ("xtb", 16)], None),
]

_compiled = {}


def _build():
    """Build + compile the per-core Bass program."""
    import concourse.mybir as mybir
    import concourse.tile as tile
    from concourse import bacc

    F32 = mybir.dt.float32
    F16 = mybir.dt.float16
    SIG = mybir.ActivationFunctionType.Sigmoid
    TANH = mybir.ActivationFunctionType.Tanh

    nc = bacc.Bacc("TRN2", target_bir_lowering=False, debug=False)

    def din(name, shape, dt):
        return nc.dram_tensor(name, shape, dt, kind="ExternalInput").ap()

    def dout(name, shape, dt):
        return nc.dram_tensor(name, shape, dt, kind="ExternalOutput").ap()

    # Transposed activations [D, NL], fp16.
    xb_l = din("xb_l", [D, NL], F16)
    xb_r = din("xb_r", [D, NL], F16)
    hb_l = din("hb_l", [D, NL], F16)
    hb_r = din("hb_r", [D, NL], F16)
    cb_l = din("cb_l", [D, NL], F16)
    cb_r = din("cb_r", [D, NL], F16)
    # z weight kt-major [Kt, P, MT, P] for the k-outer phase A; the rest
    # m-major [MT, P, Kt, P] (partition-major: per-partition DMA runs are
    # Kt*256B / MT*256B contiguous).
    wzk = din("wzk", [16, P, MT, P], F16)
    wi = din("wi", [MT, P, 40, P], F16)
    wfl = din("wfl", [MT, P, 32, P], F16)
    wfr = din("wfr", [MT, P, 32, P], F16)
    wg = din("wg", [MT, P, 24, P], F16)
    wo = din("wo", [MT, P, 32, P], F16)
    wxf = din("wxf", [MT, P, 8, P], F16)
    bias = din("bias", [P, 6, MT], F32)

    xT_o = dout("xT_o", [D, NL], F32)
    hT_o = dout("hT_o", [D, NL], F32)
    cT_o = dout("cT_o", [D, NL], F32)

    def r3(ap):
        return ap.rearrange("(k p) n -> p k n", p=P)

    with tile.TileContext(nc) as tc:
        with (
            tc.tile_pool(name="acts", bufs=1) as apool,
            tc.tile_pool(name="w", bufs=3) as wpool,
            tc.tile_pool(name="wz", bufs=5) as wzpool,
            tc.tile_pool(name="gates", bufs=8) as gpool,
            tc.tile_pool(name="work", bufs=8) as wkpool,
            tc.tile_pool(name="xst", bufs=3) as xstp,
            tc.tile_pool(name="ps", bufs=8, space="PSUM") as pspool,
            tc.tile_pool(name="cst", bufs=1) as cpool,
        ):
            bias_t = cpool.tile([P, 6, MT], F32, name="bias_t")
            nc.sync.dma_start(bias_t[:], bias[:])

            # Scratch warm-up matmuls: no DMA dependency, so they run during
            # the initial descriptor/transfer latency and lift the PE HAM
            # clock gate to 2.4GHz before real matmuls arrive.
            wu_w = cpool.tile([P, P], F16, name="wu_w")
            nc.vector.memset(wu_w[:], 0.0)
            wu_m = cpool.tile([P, 256], F16, name="wu_m")
            nc.vector.memset(wu_m[:], 0.0)
            wu_ps = pspool.tile([P, 256], F32, tag="ps", name="wu_ps")
            for _ in range(N_WARM):
                nc.tensor.matmul(wu_ps[:], wu_w[:], wu_m[:],
                                 start=True, stop=True)

            # x inputs: xbl on the HWDGE scalar ring (stores are idle early),
            # xbr on the SWDGE ring (needed only from kt=8, rides out the
            # ~4us SWDGE spin-up), one DMA per k-block in consumption order.
            xbl_t = apool.tile([P, KB, NL], F16, tag="xbl", name="xbl")
            xbr_t = apool.tile([P, KB, NL], F16, tag="xbr", name="xbr")
            for j in range(KB):
                nc.scalar.dma_start(xbl_t[:, j, :], r3(xb_l)[:, j, :])
            for j in range(KB):
                nc.gpsimd.dma_start(xbr_t[:, j, :], r3(xb_r)[:, j, :])

            # First B-gate weights prefetch on the scalar ring right behind
            # xbl, well before the sync ring finishes the z weight stream.
            w_i0a = wpool.tile([P, 24, P], F16, tag="w", name="w_i0a")
            nc.scalar.dma_start(w_i0a[:], wi[0][:, :24, :])
            w_i0b = wpool.tile([P, 16, P], F16, tag="w", name="w_i0b")
            nc.scalar.dma_start(w_i0b[:], wi[0][:, 24:, :])

            hbl_t = apool.tile([P, KB, NL], F16, tag="hbl", name="hbl")
            hbr_t = apool.tile([P, KB, NL], F16, tag="hbr", name="hbr")
            cbl_t = apool.tile([P, KB, NL], F16, tag="cbl", name="cbl")
            cbr_t = apool.tile([P, KB, NL], F16, tag="cbr", name="cbr")

            xtb_t = apool.tile([P, KB, NL], F16, tag="xtb", name="xtb")
            ctb_t = apool.tile([P, KB, NL], F16, tag="ctb", name="ctb")
            parts = {"hbl": hbl_t, "hbr": hbr_t, "cbl": cbl_t,
                     "cbr": cbr_t, "xtb": xtb_t, "ctb": ctb_t}

            # Background h/c loads, paced: each load is gated on phase-A
            # progress by a tiny gpsimd copy that reads xtb, so these 8MB
            # cannot crowd the DMA fabric during the critical first ~35us
            # (all 8 cores start cold simultaneously).
            pace_t = cpool.tile([P, 1], F16, name="pace_t")
            for gate_m, (name_, dram) in zip(
                    [0, 1, 4, 6],
                    [("hbl", hb_l), ("hbr", hb_r), ("cbl", cb_l),
                     ("cbr", cb_r)]):
                nc.gpsimd.tensor_copy(pace_t[:], xtb_t[:, gate_m, :1])
                nc.gpsimd.dma_start(parts[name_][:], r3(dram)[:])

            def chunks():
                return [slice(c * R, (c + 1) * R) for c in range(NCH)]

            # ---- Phase A: z gate + x_t ----
            # Two chunk passes, each k-outer over all 8 m-PSUM banks with a
            # ring of kt-major weight slices: the first matmul fires after
            # ~0.5MB of DMA and x streams in k-block order at ~150GB/s.
            for ci, cs in enumerate(chunks()):
                pss = [pspool.tile([P, R], F32, tag="ps", name=f"ps_z{m}")
                       for m in range(MT)]
                for kt in range(16):
                    wslc = wzpool.tile([P, MT, P], F16, tag="wz",
                                       name="wz_slc")
                    nc.sync.dma_start(wslc[:], wzk[kt])
                    xsrc = (xbl_t if kt < KB else xbr_t)[:, kt % KB, cs]
                    for m in range(MT):
                        nc.tensor.matmul(pss[m][:], wslc[:, m, :], xsrc,
                                         start=(kt == 0), stop=(kt == 15))
                for m in range(MT):
                    z_t = wkpool.tile([P, R], F32, tag="wk", name="z_t")
                    nc.scalar.activation(z_t[:], pss[m][:], SIG,
                                         bias=bias_t[:, 0, m, None])
                    d_t = wkpool.tile([P, R], F32, tag="wk", name="d_t")
                    nc.vector.tensor_sub(d_t[:], xbl_t[:, m, cs],
                                         xbr_t[:, m, cs])
                    xrf_m = wkpool.tile([P, R], F32, tag="wk", name="xrf_m")
                    nc.vector.tensor_copy(xrf_m[:], xbr_t[:, m, cs])
                    nc.vector.tensor_mul(d_t[:], d_t[:], z_t[:])
                    xt_m = xstp.tile([P, R], F32, tag="xst", name="xt_m")
                    nc.vector.tensor_add(xt_m[:], d_t[:], xrf_m[:])
                    nc.scalar.dma_start(r3(xT_o)[:, m, cs], xt_m[:])
                    nc.vector.tensor_copy(xtb_t[:, m, cs], xt_m[:])

            # ---- Phase B: i, f_l, f_r, g gates + c_t ----
            # The i-gate weight loads in two pieces so the w tag ring slot
            # stays at 8KB/partition (SBUF budget).
            for m in range(MT):
                gt = {}   # gate -> per-chunk fp32 tiles
                xfp = []
                for (gname, Kt, b_idx, fn, rparts, xkey) in _GATES_B:
                    if gname == "i":
                        if m == 0:
                            w_a, w_b = w_i0a, w_i0b
                        else:
                            w_a = wpool.tile([P, 24, P], F16, tag="w",
                                             name="w_ia")
                            nc.sync.dma_start(w_a[:], wi[m][:, :24, :])
                            w_b = wpool.tile([P, 16, P], F16, tag="w",
                                             name="w_ib")
                            nc.sync.dma_start(w_b[:], wi[m][:, 24:, :])

                        def wsl(kt):
                            return (w_a[:, kt, :] if kt < 24
                                    else w_b[:, kt - 24, :])
                    else:
                        dram = {"xf": wxf, "fl": wfl, "fr": wfr,
                                "g": wg}[gname]
                        w_t = wpool.tile([P, Kt, P], F16, tag="w",
                                         name=f"w_{gname}")
                        nc.sync.dma_start(w_t[:], dram[m])

                        def wsl(kt, w_t=w_t):
                            return w_t[:, kt, :]
                    per_chunk = []
                    for cs in chunks():
                        ps = pspool.tile([P, R], F32, tag="ps",
                                         name=f"ps_{gname}")
                        n_done = 0
                        for (pname, koff) in rparts:
                            pt = parts[pname]
                            for j in range(KB):
                                nc.tensor.matmul(
                                    ps[:], wsl(koff + j), pt[:, j, cs],
                                    start=(n_done == 0),
                                    stop=(n_done == Kt - 1))
                                n_done += 1
                        if gname == "xf":
                            xf_c = gpool.tile([P, R], F32, tag="gate",
                                              name="xfp")
                            nc.scalar.copy(xf_c[:], ps[:])
                            xfp.append(xf_c)
                            continue
                        if xkey == "xf":
                            nc.vector.tensor_add(ps[:], ps[:],
                                                 xfp[len(per_chunk)][:])
                        g_t = gpool.tile([P, R], F32, tag="gate",
                                         name=f"g_{gname}")
                        nc.scalar.activation(
                            g_t[:], ps[:], SIG if fn == "sig" else TANH,
                            bias=bias_t[:, b_idx, m, None])
                        per_chunk.append(g_t)
                    if gname != "xf":
                        gt[gname] = per_chunk
                for ci, cs in enumerate(chunks()):
                    cfl_m = wkpool.tile([P, R], F32, tag="wk", name="cfl_m")
                    nc.vector.tensor_copy(cfl_m[:], cbl_t[:, m, cs])
                    cfr_m = wkpool.tile([P, R], F32, tag="wk", name="cfr_m")
                    nc.vector.tensor_copy(cfr_m[:], cbr_t[:, m, cs])
                    ct_m = wkpool.tile([P, R], F32, tag="wk", name="ct_m")
                    nc.vector.tensor_mul(ct_m[:], gt["fl"][ci][:], cfl_m[:])
                    t2 = wkpool.tile([P, R], F32, tag="wk", name="t2")
                    nc.vector.tensor_mul(t2[:], gt["fr"][ci][:], cfr_m[:])
                    nc.vector.tensor_add(ct_m[:], ct_m[:], t2[:])
                    nc.vector.tensor_mul(t2[:], gt["i"][ci][:], gt["g"][ci][:])
                    nc.vector.tensor_add(ct_m[:], ct_m[:], t2[:])
                    nc.scalar.dma_start(r3(cT_o)[:, m, cs], ct_m[:])
                    nc.vector.tensor_copy(ctb_t[:, m, cs], ct_m[:])

            # ---- Phase C: o gate + h_t ---- (ctb last in K so phase B's
            # tail hides under the hbl/hbr/xtb matmuls)
            o_parts = [("hbl", 0), ("hbr", 8), ("xtb", 24), ("ctb", 16)]
            tct7 = cpool.tile([P, NCH, R], F32, name="tct7")
            for m in range(MT):
                w_t = wpool.tile([P, 32, P], F16, tag="w", name="wo_t")
                nc.sync.dma_start(w_t[:], wo[m])
                last = (m == MT - 1)
                for ci, cs in enumerate(chunks()):
                    # The very last chunk runs in 256-col halves so its
                    # activation/store chain overlaps its own matmuls.
                    halves = ([slice(0, 256), slice(256, 512)]
                              if (last and ci == NCH - 1) else [slice(0, R)])
                    for hs in halves:
                        rh = hs.stop - hs.start
                        rs = slice(cs.start + hs.start, cs.start + hs.stop)
                        ps = pspool.tile([P, rh], F32, tag="ps", name="ps_o")
                        kt = 0
                        for pname, koff in o_parts:
                            pt = parts[pname]
                            for j in range(KB):
                                nc.tensor.matmul(ps[:], w_t[:, koff + j, :],
                                                 pt[:, j, rs],
                                                 start=(kt == 0),
                                                 stop=(kt == 31))
                                kt += 1
                        o_t = wkpool.tile([P, rh], F32, tag="wk", name="o_t")
                        nc.scalar.activation(o_t[:], ps[:], SIG,
                                             bias=bias_t[:, 5, m, None])
                        if last:
                            tct_v = tct7[:, ci, hs]
                        else:
                            tct_m = wkpool.tile([P, rh], F32, tag="wk",
                                                name="tct_m")
                            nc.scalar.activation(tct_m[:], ctb_t[:, m, rs],
                                                 TANH)
                            tct_v = tct_m[:]
                        ht_m = wkpool.tile([P, rh], F32, tag="wk", name="ht_m")
                        nc.vector.tensor_mul(ht_m[:], o_t[:], tct_v)
                        nc.scalar.dma_start(r3(hT_o)[:, m, rs], ht_m[:])
                if m == 0:
                    # Precompute the last m-tile's tanh(c) while its ctb is
                    # long since ready, off the critical tail.
                    for ci, cs in enumerate(chunks()):
                        nc.scalar.activation(tct7[:, ci, :],
                                             ctb_t[:, MT - 1, cs], TANH)

    nc.compile()
    return nc


def _get_compiled():
    if "k" not in _compiled:
        _compiled["k"] = _build()
    return _compiled["k"]


def _prep_weight(w_km):
    """[K, D] (K-major stack of W.T blocks) -> [MT, P, Kt, P] fp16."""
    K = w_km.shape[0]
    kt = K // P
    w = w_km.reshape(kt, P, MT, P)          # [kt, p, m, f]
    w = np.ascontiguousarray(w.transpose(2, 1, 0, 3))  # [m, p, kt, f]
    return w.astype(np.float16)


def _host_prep(inp):
    """Transpose/stack/cast everything the device program wants."""
    f32 = np.float32
    t = {k: np.asarray(inp[k], dtype=f32).T.astype(np.float16)
         for k in ("x_l", "x_r", "h_l", "h_r", "c_l", "c_r")}

    W_i = np.asarray(inp["W_i"], f32)
    W_fl = np.asarray(inp["W_fl"], f32)
    W_fr = np.asarray(inp["W_fr"], f32)
    W_xin = np.asarray(inp["W_xin"], f32)
    W_o = np.asarray(inp["W_o"], f32)
    W_z = np.asarray(inp["W_z"], f32)
    W_g = np.asarray(inp["W_g"], f32)

    x_i = W_xin[0 * D:1 * D].T    # [D, D] blocks of W_xin.T
    x_f = W_xin[1 * D:2 * D].T
    x_o = W_xin[2 * D:3 * D].T
    x_g = W_xin[3 * D:4 * D].T

    # kt-major z weight for the k-outer phase A: [Kt, P, MT, P].
    wz_km = np.ascontiguousarray(W_z.T)               # [2048, 1024]
    wzk = np.ascontiguousarray(wz_km.reshape(16, P, MT, P)).astype(np.float16)

    # Device-side part offsets:
    #   wi: [h_l h_r c_l c_r | x_i]  (hbl 0, hbr 8, cbl 16, cbr 24, xtb 32)
    #   wg: [h_l h_r | x_g]          (hbl 0, hbr 8, xtb 16)
    #   wo: [h_l h_r c_t | x_o]      (hbl 0, hbr 8, ctb 16, xtb 24)
    weights = {
        "wzk": wzk,
        "wi": _prep_weight(np.concatenate([W_i.T, x_i], axis=0)),
        "wfl": _prep_weight(np.ascontiguousarray(W_fl.T)),
        "wfr": _prep_weight(np.ascontiguousarray(W_fr.T)),
        "wg": _prep_weight(np.concatenate([W_g.T, x_g], axis=0)),
        "wo": _prep_weight(np.concatenate([W_o.T, x_o], axis=0)),
        "wxf": _prep_weight(x_f),
    }

    b = np.stack([np.asarray(inp[k], f32) for k in
                  ("b_z", "b_i", "b_fl", "b_fr", "b_g", "b_o")])  # [6, D]
    bias = np.ascontiguousarray(b.reshape(6, MT, P).transpose(2, 0, 1))

    in_maps = []
    for c in range(N_CORES):
        cs = slice(c * NL, (c + 1) * NL)
        m = {
            "xb_l": np.ascontiguousarray(t["x_l"][:, cs]),
            "xb_r": np.ascontiguousarray(t["x_r"][:, cs]),
            "hb_l": np.ascontiguousarray(t["h_l"][:, cs]),
            "hb_r": np.ascontiguousarray(t["h_r"][:, cs]),
            "cb_l": np.ascontiguousarray(t["c_l"][:, cs]),
            "cb_r": np.ascontiguousarray(t["c_r"][:, cs]),
            "bias": bias,
        }
        m.update(weights)
        in_maps.append(m)
    return in_maps


def run(inputs, trace=False, trace_kwargs=None):
    """Run on 8 cores; returns (results, BassKernelResults)."""
    from concourse.bass_utils import run_bass_kernel_spmd

    if trace:
        try:
            from hookfix import install_ntff_hook
            install_ntff_hook()
        except Exception:
            pass
    nc = _get_compiled()
    in_maps = _host_prep(inputs)
    res = run_bass_kernel_spmd(nc, in_maps, core_ids=list(range(N_CORES)),
                               trace=trace, **(trace_kwargs or {}))
    xT = np.concatenate([res.results[c]["xT_o"] for c in range(N_CORES)], axis=1)
    hT = np.concatenate([res.results[c]["hT_o"] for c in range(N_CORES)], axis=1)
    cT = np.concatenate([res.results[c]["cT_o"] for c in range(N_CORES)], axis=1)
    x_t = np.ascontiguousarray(xT.T)
    h_t = np.ascontiguousarray(hT.T)
    c_t = np.ascontiguousarray(cT.T)
    return (x_t, h_t, c_t), res


def kernel(**inputs):
    out, _ = run(inputs)
    return out


# revision 14
# speedup vs baseline: 1.0521x; 1.0261x over previous
"""Trainium2 Bass kernel for a batched binary-tree (child-sum-ish) LSTM cell.

Computes, for N=8192 nodes (D = HD = 1024):
    z   = sigmoid([x_l x_r] @ W_z.T + b_z)
    x_t = z * x_l + (1-z) * x_r
    [x_i x_f x_o x_g] = x_t @ W_xin.T
    i   = sigmoid([h_l h_r c_l c_r] @ W_i.T  + b_i  + x_i)
    f_l = sigmoid([h_l h_r c_l c_r] @ W_fl.T + b_fl + x_f)
    f_r = sigmoid([h_l h_r c_l c_r] @ W_fr.T + b_fr + x_f)
    g   = tanh   ([h_l h_r]         @ W_g.T  + b_g  + x_g)
    c_t = f_l*c_l + f_r*c_r + i*g
    o   = sigmoid([h_l h_r c_t]     @ W_o.T  + b_o  + x_o)
    h_t = o * tanh(c_t)
returns (x_t, h_t, c_t).

Strategy: data-parallel over 8 NeuronCores (NL=1024 rows each), transposed
space (features on partitions, rows on the free dim). All activations stay
SBUF-resident, processed as two 512-row chunks (PSUM bank = 512 fp32), so
every weight tile streams from HBM once (z weight twice, once per chunk
pass). Matmuls in fp16 (8x less rounding than bf16, same PE speed), fp32
PSUM/elementwise.

Ramp engineering: all 8 cores start pulling HBM at t=0, so the chip-wide
DMA fabric is the scarce resource for the first ~30us. Phase A (z) runs
k-outer over all 8 m-PSUMs per chunk pass, consuming x and the kt-major z
weight slices in stream order at ~300GB/s; everything not needed in that
window (h, c loads) is gated behind phase-A progress via tiny gpsimd
copies, so it cannot crowd the critical stream. A short scratch-matmul
burst bridges the initial DMA latency and lifts the PE HAM clock gate.
Cross-phase inputs (xtb, ctb) sit late in each gate's K-accumulation so
phase tails hide under dependency-free matmuls; the last output tile is
processed in 256-column halves to overlap its activation/store chain.
"""

import sys

if "/opt/trn_rl_repo" not in sys.path:
    sys.path.insert(0, "/opt/trn_rl_repo")

import numpy as np

N_CORES = 8
N = 8192
D = 1024
P = 128
NL = N // N_CORES          # rows per core
KB = D // P                # 8 k-blocks per 1024-feature tensor
MT = D // P                # 8 output m-tiles per gate
R = 512                    # rows per matmul chunk (PSUM bank = 512 fp32)
NCH = NL // R              # 2 chunks
N_WARM = 16                # scratch matmuls to ramp the HAM clock gate

# (name, K-tiles, bias index, act fn, [(rhs part, weight k-tile offset)]).
# Part order: h's first (streaming in), xtb mid (hides phase A's tail), c's
# late (their DMA trails the h's).
_GATES_B = [
    ("i",  40, 1, "sig",  [("hbl", 0), ("hbr", 8), ("xtb", 32),
                           ("cbl", 16), ("cbr", 24)], None),
    ("xf", 8,  None, None, [("xtb", 0)], None),
    ("fl", 32, 2, "sig",  [("hbl", 0), ("hbr", 8), ("cbl", 16), ("cbr", 24)], "xf"),
    ("fr", 32, 3, "sig",  [("hbl", 0), ("hbr", 8), ("cbl", 16), ("cbr", 24)], "xf"),
    ("g",  24, 4, "tanh", [("hbl", 0), ("hbr", 8), ("xtb", 16)], None),
]

_compiled = {}


def _build():
    """Build + compile the per-core Bass program."""
    import concourse.mybir as mybir
    import concourse.tile as tile
    from concourse import bacc

    F32 = mybir.dt.float32
    F16 = mybir.dt.float16
    SIG = mybir.ActivationFunctionType.Sigmoid
    TANH = mybir.ActivationFunctionType.Tanh

    nc = bacc.Bacc("TRN2", target_bir_lowering=False, debug=False)

    def din(name, shape, dt):
        return nc.dram_tensor(name, shape, dt, kind="ExternalInput").ap()

    def dout(name, shape, dt):
        return nc.dram_tensor(name, shape, dt, kind="ExternalOutput").ap()

    # Transposed activations [D, NL], fp16.
    xb_l = din("xb_l", [D, NL], F16)
    xb_r = din("xb_r", [D, NL], F16)
    hb_l = din("hb_l", [D, NL], F16)
    hb_r = din("hb_r", [D, NL], F16)
    cb_l = din("cb_l", [D, NL], F16)
    cb_r = din("cb_r", [D, NL], F16)
    # z weight kt-major [Kt, P, MT, P] for the k-outer phase A; the rest
    # m-major [MT, P, Kt, P] (partition-major: per-partition DMA runs are
    # Kt*256B / MT*256B contiguous).
    wzk = din("wzk", [16, P, MT, P], F16)
    wi = din("wi", [MT, P, 40, P], F16)
    wfl = din("wfl", [MT, P, 32, P], F16)
    wfr = din("wfr", [MT, P, 32, P], F16)
    wg = din("wg", [MT, P, 24, P], F16)
    wo = din("wo", [MT, P, 32, P], F16)
    wxf = din("wxf", [MT, P, 8, P], F16)
    bias = din("bias", [P, 6, MT], F32)

    xT_o = dout("xT_o", [D, NL], F32)
    hT_o = dout("hT_o", [D, NL], F32)
    cT_o = dout("cT_o", [D, NL], F32)

    def r3(ap):
        return ap.rearrange("(k p) n -> p k n", p=P)

    with tile.TileContext(nc) as tc:
        with (
            tc.tile_pool(name="acts", bufs=1) as apool,
            tc.tile_pool(name="w", bufs=3) as wpool,
            tc.tile_pool(name="wz", bufs=5) as wzpool,
            tc.tile_pool(name="gates", bufs=8) as gpool,
            tc.tile_pool(name="work", bufs=8) as wkpool,
            tc.tile_pool(name="xst", bufs=3) as xstp,
            tc.tile_pool(name="ps", bufs=8, space="PSUM") as pspool,
            tc.tile_pool(name="cst", bufs=1) as cpool,
        ):
            bias_t = cpool.tile([P, 6, MT], F32, name="bias_t")
            nc.sync.dma_start(bias_t[:], bias[:])

            # Scratch warm-up matmuls: no DMA dependency, so they run during
            # the initial descriptor/transfer latency and lift the PE HAM
            # clock gate to 2.4GHz before real matmuls arrive.
            wu_w = cpool.tile([P, P], F16, name="wu_w")
            nc.vector.memset(wu_w[:], 0.0)
            wu_m = cpool.tile([P, 256], F16, name="wu_m")
            nc.vector.memset(wu_m[:], 0.0)
            wu_ps = pspool.tile([P, 256], F32, tag="ps", name="wu_ps")
            for _ in range(N_WARM):
                nc.tensor.matmul(wu_ps[:], wu_w[:], wu_m[:],
                                 start=True, stop=True)

            # x inputs split across both HWDGE-triggered queues (scalar +
            # gpsimd) in consumption order, so neither queue has to sustain
            # the full 145GB/s k-block stream alone during the cold ramp.
            # The SWDGE spin-up (~4us) is hidden by giving gpsimd only
            # blocks needed from the 3rd kt-round on.
            xbl_t = apool.tile([P, KB, NL], F16, tag="xbl", name="xbl")
            xbr_t = apool.tile([P, KB, NL], F16, tag="xbr", name="xbr")
            for b in [0, 1, 3, 5, 7, 8, 10, 12, 14]:
                t, j = (xbl_t, b) if b < KB else (xbr_t, b - KB)
                src = r3(xb_l if b < KB else xb_r)
                nc.scalar.dma_start(t[:, j, :], src[:, j, :])
            for b in [2, 4, 6, 9, 11, 13, 15]:
                t, j = (xbl_t, b) if b < KB else (xbr_t, b - KB)
                src = r3(xb_l if b < KB else xb_r)
                nc.gpsimd.dma_start(t[:, j, :], src[:, j, :])

            # B-m0 gate weights prefetch on the scalar ring behind x.
            # Alloc order (xf0, g0, i0a, i0b) keeps the 3-slot w ring
            # acyclic: i0b lands in xf0's slot and xf runs first at B-m0.
            w_xf0 = wpool.tile([P, 8, P], F16, tag="w", name="w_xf0")
            nc.scalar.dma_start(w_xf0[:], wxf[0])
            w_g0 = wpool.tile([P, 24, P], F16, tag="w", name="w_g0")
            nc.scalar.dma_start(w_g0[:], wg[0])
            w_i0a = wpool.tile([P, 24, P], F16, tag="w", name="w_i0a")
            nc.scalar.dma_start(w_i0a[:], wi[0][:, :24, :])
            w_i0b = wpool.tile([P, 16, P], F16, tag="w", name="w_i0b")
            nc.scalar.dma_start(w_i0b[:], wi[0][:, 24:, :])

            hbl_t = apool.tile([P, KB, NL], F16, tag="hbl", name="hbl")
            hbr_t = apool.tile([P, KB, NL], F16, tag="hbr", name="hbr")
            cbl_t = apool.tile([P, KB, NL], F16, tag="cbl", name="cbl")
            cbr_t = apool.tile([P, KB, NL], F16, tag="cbr", name="cbr")

            xtb_t = apool.tile([P, KB, NL], F16, tag="xtb", name="xtb")
            ctb_t = apool.tile([P, KB, NL], F16, tag="ctb", name="ctb")
            parts = {"hbl": hbl_t, "hbr": hbr_t, "cbl": cbl_t,
                     "cbr": cbr_t, "xtb": xtb_t, "ctb": ctb_t}

            # Background h/c loads, paced: each load is gated on phase-A
            # progress by a tiny gpsimd copy that reads xtb, so these 8MB
            # cannot crowd the DMA fabric during the critical first ~35us
            # (all 8 cores start cold simultaneously).
            pace_t = cpool.tile([P, 1], F16, name="pace_t")
            for gate_m, (name_, dram) in zip(
                    [0, 1, 2, 3],
                    [("hbl", hb_l), ("hbr", hb_r), ("cbl", cb_l),
                     ("cbr", cb_r)]):
                nc.gpsimd.tensor_copy(pace_t[:], xtb_t[:, gate_m, :1])
                nc.gpsimd.dma_start(parts[name_][:], r3(dram)[:])

            def chunks():
                return [slice(c * R, (c + 1) * R) for c in range(NCH)]

            # ---- Phase A: z gate + x_t ----
            # Two groups of 4 m-tiles; each group runs k-outer with both
            # 512-row chunks inside the kt round (4m x 2ch = 8 PSUM banks),
            # so x is consumed exactly once at a smooth ~145GB/s and each
            # group's kt-major z weight columns stream once (~76GB/s).
            MG = 4
            for g in range(MT // MG):
                mg = [g * MG + i for i in range(MG)]
                pss = [[pspool.tile([P, R], F32, tag="ps",
                                    name=f"ps_z{m}{c}") for c in range(NCH)]
                       for m in mg]
                for kt in range(16):
                    wslc = wzpool.tile([P, MG, P], F16, tag="wz",
                                       name="wz_slc")
                    nc.sync.dma_start(
                        wslc[:], wzk[kt][:, g * MG:(g + 1) * MG, :])
                    xsrc = (xbl_t if kt < KB else xbr_t)[:, kt % KB, :]
                    for mi in range(MG):
                        for ci, cs in enumerate(chunks()):
                            nc.tensor.matmul(pss[mi][ci][:], wslc[:, mi, :],
                                             xsrc[:, cs],
                                             start=(kt == 0), stop=(kt == 15))
                for mi in range(MG):
                    m = mg[mi]
                    for ci, cs in enumerate(chunks()):
                        z_t = wkpool.tile([P, R], F32, tag="wk", name="z_t")
                        nc.scalar.activation(z_t[:], pss[mi][ci][:], SIG,
                                             bias=bias_t[:, 0, m, None])
                        d_t = wkpool.tile([P, R], F32, tag="wk", name="d_t")
                        nc.vector.tensor_sub(d_t[:], xbl_t[:, m, cs],
                                             xbr_t[:, m, cs])
                        xrf_m = wkpool.tile([P, R], F32, tag="wk",
                                            name="xrf_m")
                        nc.vector.tensor_copy(xrf_m[:], xbr_t[:, m, cs])
                        nc.vector.tensor_mul(d_t[:], d_t[:], z_t[:])
                        xt_m = xstp.tile([P, R], F32, tag="xst", name="xt_m")
                        nc.vector.tensor_add(xt_m[:], d_t[:], xrf_m[:])
                        nc.scalar.dma_start(r3(xT_o)[:, m, cs], xt_m[:])
                        nc.vector.tensor_copy(xtb_t[:, m, cs], xt_m[:])

            # ---- Phase B: i, f_l, f_r, g gates + c_t ----
            # The i-gate weight loads in two pieces so the w tag ring slot
            # stays at 8KB/partition (SBUF budget).
            for m in range(MT):
                gt = {}   # gate -> per-chunk fp32 tiles
                xfp = []
                # For m=0 the h-only/x-only gates run first so the c loads
                # (paced behind phase A) get ~14us more arrival slack.
                order = ([1, 4, 0, 2, 3] if m == 0 else [0, 1, 2, 3, 4])
                for gi in order:
                    (gname, Kt, b_idx, fn, rparts, xkey) = _GATES_B[gi]
                    if gname == "i":
                        if m == 0:
                            w_a, w_b = w_i0a, w_i0b
                        else:
                            w_a = wpool.tile([P, 24, P], F16, tag="w",
                                             name="w_ia")
                            nc.sync.dma_start(w_a[:], wi[m][:, :24, :])
                            w_b = wpool.tile([P, 16, P], F16, tag="w",
                                             name="w_ib")
                            nc.sync.dma_start(w_b[:], wi[m][:, 24:, :])

                        def wsl(kt):
                            return (w_a[:, kt, :] if kt < 24
                                    else w_b[:, kt - 24, :])
                    elif m == 0 and gname in ("xf", "g"):
                        w_t = w_xf0 if gname == "xf" else w_g0

                        def wsl(kt, w_t=w_t):
                            return w_t[:, kt, :]
                    else:
                        dram = {"xf": wxf, "fl": wfl, "fr": wfr,
                                "g": wg}[gname]
                        w_t = wpool.tile([P, Kt, P], F16, tag="w",
                                         name=f"w_{gname}")
                        nc.sync.dma_start(w_t[:], dram[m])

                        def wsl(kt, w_t=w_t):
                            return w_t[:, kt, :]
                    per_chunk = []
                    for cs in chunks():
                        ps = pspool.tile([P, R], F32, tag="ps",
                                         name=f"ps_{gname}")
                        n_done = 0
                        for (pname, koff) in rparts:
                            pt = parts[pname]
                            for j in range(KB):
                                nc.tensor.matmul(
                                    ps[:], wsl(koff + j), pt[:, j, cs],
                                    start=(n_done == 0),
                                    stop=(n_done == Kt - 1))
                                n_done += 1
                        if gname == "xf":
                            xf_c = gpool.tile([P, R], F32, tag="gate",
                                              name="xfp")
                            nc.scalar.copy(xf_c[:], ps[:])
                            xfp.append(xf_c)
                            continue
                        if xkey == "xf":
                            nc.vector.tensor_add(ps[:], ps[:],
                                                 xfp[len(per_chunk)][:])
                        g_t = gpool.tile([P, R], F32, tag="gate",
                                         name=f"g_{gname}")
                        nc.scalar.activation(
                            g_t[:], ps[:], SIG if fn == "sig" else TANH,
                            bias=bias_t[:, b_idx, m, None])
                        per_chunk.append(g_t)
                    if gname != "xf":
                        gt[gname] = per_chunk
                for ci, cs in enumerate(chunks()):
                    cfl_m = wkpool.tile([P, R], F32, tag="wk", name="cfl_m")
                    nc.vector.tensor_copy(cfl_m[:], cbl_t[:, m, cs])
                    cfr_m = wkpool.tile([P, R], F32, tag="wk", name="cfr_m")
                    nc.vector.tensor_copy(cfr_m[:], cbr_t[:, m, cs])
                    ct_m = wkpool.tile([P, R], F32, tag="wk", name="ct_m")
                    nc.vector.tensor_mul(ct_m[:], gt["fl"][ci][:], cfl_m[:])
                    t2 = wkpool.tile([P, R], F32, tag="wk", name="t2")
                    nc.vector.tensor_mul(t2[:], gt["fr"][ci][:], cfr_m[:])
                    nc.vector.tensor_add(ct_m[:], ct_m[:], t2[:])
                    nc.vector.tensor_mul(t2[:], gt["i"][ci][:], gt["g"][ci][:])
                    nc.vector.tensor_add(ct_m[:], ct_m[:], t2[:])
                    nc.scalar.dma_start(r3(cT_o)[:, m, cs], ct_m[:])
                    nc.vector.tensor_copy(ctb_t[:, m, cs], ct_m[:])

            # ---- Phase C: o gate + h_t ---- (ctb last in K so phase B's
            # tail hides under the hbl/hbr/xtb matmuls)
            o_parts = [("hbl", 0), ("hbr", 8), ("xtb", 24), ("ctb", 16)]
            tct7 = cpool.tile([P, NCH, R], F32, name="tct7")
            for m in range(MT):
                w_t = wpool.tile([P, 32, P], F16, tag="w", name="wo_t")
                nc.sync.dma_start(w_t[:], wo[m])
                last = (m == MT - 1)
                for ci, cs in enumerate(chunks()):
                    # The very last chunk runs in 256-col halves so its
                    # activation/store chain overlaps its own matmuls.
                    halves = ([slice(0, 256), slice(256, 512)]
                              if (last and ci == NCH - 1) else [slice(0, R)])
                    for hs in halves:
                        rh = hs.stop - hs.start
                        rs = slice(cs.start + hs.start, cs.start + hs.stop)
                        ps = pspool.tile([P, rh], F32, tag="ps", name="ps_o")
                        kt = 0
                        for pname, koff in o_parts:
                            pt = parts[pname]
                            for j in range(KB):
                                nc.tensor.matmul(ps[:], w_t[:, koff + j, :],
                                                 pt[:, j, rs],
                                                 start=(kt == 0),
                                                 stop=(kt == 31))
                                kt += 1
                        o_t = wkpool.tile([P, rh], F32, tag="wk", name="o_t")
                        nc.scalar.activation(o_t[:], ps[:], SIG,
                                             bias=bias_t[:, 5, m, None])
                        if last:
                            tct_v = tct7[:, ci, hs]
                        else:
                            tct_m = wkpool.tile([P, rh], F32, tag="wk",
                                                name="tct_m")
                            nc.scalar.activation(tct_m[:], ctb_t[:, m, rs],
                                                 TANH)
                            tct_v = tct_m[:]
                        ht_m = wkpool.tile([P, rh], F32, tag="wk", name="ht_m")
                        nc.vector.tensor_mul(ht_m[:], o_t[:], tct_v)
                        nc.scalar.dma_start(r3(hT_o)[:, m, rs], ht_m[:])
                if m == 0:
                    # Precompute the last m-tile's tanh(c) while its ctb is
                    # long since ready, off the critical tail.
                    for ci, cs in enumerate(chunks()):
                        nc.scalar.activation(tct7[:, ci, :],
                                             ctb_t[:, MT - 1, cs], TANH)

    nc.compile()
    return nc


def _get_compiled():
    if "k" not in _compiled:
        _compiled["k"] = _build()
    return _compiled["k"]


def _prep_weight(w_km):
    """[K, D] (K-major stack of W.T blocks) -> [MT, P, Kt, P] fp16."""
    K = w_km.shape[0]
    kt = K // P
    w = w_km.reshape(kt, P, MT, P)          # [kt, p, m, f]
    w = np.ascontiguousarray(w.transpose(2, 1, 0, 3))  # [m, p, kt, f]
    return w.astype(np.float16)


def _host_prep(inp):
    """Transpose/stack/cast everything the device program wants."""
    f32 = np.float32
    t = {k: np.asarray(inp[k], dtype=f32).T.astype(np.float16)
         for k in ("x_l", "x_r", "h_l", "h_r", "c_l", "c_r")}

    W_i = np.asarray(inp["W_i"], f32)
    W_fl = np.asarray(inp["W_fl"], f32)
    W_fr = np.asarray(inp["W_fr"], f32)
    W_xin = np.asarray(inp["W_xin"], f32)
    W_o = np.asarray(inp["W_o"], f32)
    W_z = np.asarray(inp["W_z"], f32)
    W_g = np.asarray(inp["W_g"], f32)

    x_i = W_xin[0 * D:1 * D].T    # [D, D] blocks of W_xin.T
    x_f = W_xin[1 * D:2 * D].T
    x_o = W_xin[2 * D:3 * D].T
    x_g = W_xin[3 * D:4 * D].T

    # kt-major z weight for the k-outer phase A: [Kt, P, MT, P].
    wz_km = np.ascontiguousarray(W_z.T)               # [2048, 1024]
    wzk = np.ascontiguousarray(wz_km.reshape(16, P, MT, P)).astype(np.float16)

    # Device-side part offsets:
    #   wi: [h_l h_r c_l c_r | x_i]  (hbl 0, hbr 8, cbl 16, cbr 24, xtb 32)
    #   wg: [h_l h_r | x_g]          (hbl 0, hbr 8, xtb 16)
    #   wo: [h_l h_r c_t | x_o]      (hbl 0, hbr 8, ctb 16, xtb 24)
    weights = {
        "wzk": wzk,
        "wi": _prep_weight(np.concatenate([W_i.T, x_i], axis=0)),
        "wfl": _prep_weight(np.ascontiguousarray(W_fl.T)),
        "wfr": _prep_weight(np.ascontiguousarray(W_fr.T)),
        "wg": _prep_weight(np.concatenate([W_g.T, x_g], axis=0)),
        "wo": _prep_weight(np.concatenate([W_o.T, x_o], axis=0)),
        "wxf": _prep_weight(x_f),
    }

    b = np.stack([np.asarray(inp[k], f32) for k in
                  ("b_z", "b_i", "b_fl", "b_fr", "b_g", "b_o")])  # [6, D]
    bias = np.ascontiguousarray(b.reshape(6, MT, P).transpose(2, 0, 1))

    in_maps = []
    for c in range(N_CORES):
        cs = slice(c * NL, (c + 1) * NL)
        m = {
            "xb_l": np.ascontiguousarray(t["x_l"][:, cs]),
            "xb_r": np.ascontiguousarray(t["x_r"][:, cs]),
            "hb_l": np.ascontiguousarray(t["h_l"][:, cs]),
            "hb_r": np.ascontiguousarray(t["h_r"][:, cs]),
            "cb_l": np.ascontiguousarray(t["c_l"][:, cs]),
            "cb_r": np.ascontiguousarray(t["c_r"][:, cs]),
            "bias": bias,
        }
        m.update(weights)
        in_maps.append(m)
    return in_maps


def run(inputs, trace=False, trace_kwargs=None):
    """Run on 8 cores; returns (results, BassKernelResults)."""
    from concourse.bass_utils import run_bass_kernel_spmd

    if trace:
        try:
            from hookfix import install_ntff_hook
            install_ntff_hook()
        except Exception:
            pass
    nc = _get_compiled()
    in_maps = _host_prep(inputs)
    res = run_bass_kernel_spmd(nc, in_maps, core_ids=list(range(N_CORES)),
                               trace=trace, **(trace_kwargs or {}))
    xT = np.concatenate([res.results[c]["xT_o"] for c in range(N_CORES)], axis=1)
    hT = np.concatenate([res.results[c]["hT_o"] for c in range(N_CORES)], axis=1)
    cT = np.concatenate([res.results[c]["cT_o"] for c in range(N_CORES)], axis=1)
    x_t = np.ascontiguousarray(xT.T)
    h_t = np.ascontiguousarray(hT.T)
    c_t = np.ascontiguousarray(cT.T)
    return (x_t, h_t, c_t), res


def kernel(**inputs):
    out, _ = run(inputs)
    return out
